# revision 17
# baseline (speedup 1.0000x reference)
"""GQA kernel for 8 TRN2 NeuronCores.

Model: B=4, T=2048, C=1024, 16 q heads / 4 kv heads / head_dim 64, causal.
Sharding: 16 (batch, kv-head-pair) units -> 2 per core. Core c handles batch
c//2 and kv-head pair (0,1) if c even else (2,3), i.e. q heads 0-7 or 8-15.
Each core computes its slice of the Q/K/V projections, local causal
attention, and a partial output projection (its 512 columns of the head
concat); the host sums the two partial y's per batch.

v6 (from ~305us v3 baseline; v4 = 283us):
- all weights + constants land in two DMAs (first-needed bytes first)
  instead of ~30 serial dma_starts (~0.6-1us fixed cost each).
- causal diagonal masking split DVE (r<2, tensor_mul with a 0/1 triangle
  tile, ~0.3us) / gpsimd affine_select (r>=2, ~1.1us) instead of all-gpsimd
  (134us busy in v3).
- r=3 score matmuls restricted to their live 128 query columns.
- V tiles in bf16 (PE transpose at 1 cycle/row).
- full-tile exps issued on flattened [128,1024] APs -> one ACT inst each.
- kT / vext split into PER-BLOCK tiles and stage emission reordered to
  B(t) -> A(t+1) -> C(t): A(t+1)'s writes no longer carry a whole-tile WAR
  hazard against B(t)'s reads, so the projection matmuls can fill the PE
  while B(t) is Scalar(exp)-bound.

Fused software pipeline over token blocks of 512: A(t) projections for
token block t -> B(c=t) attention for query block c (needs only k/v token
blocks <= c by causality) -> C(c) output projection rows for block c.

Attention per (pair p, query block c): head pair shares one qT tile
([128, 512] x 4: rows 0-63 head A, 64-127 head B). Scores for A and B are
two concurrent row-tiled K=64 matmuls into one 2-bank PSUM tile; one Exp
activation covers both. PV uses Vext=[V|1] (M=65) so the softmax
denominator falls out of row 64 of the PV accumulator; reciprocal via the
fast approx DVE op (~51 ULP) after a [1,512] copy to SBUF (custom DVE ops
can't read PSUM), then gpsimd partition_broadcast (dst must start at
partition 0).
"""

import numpy as np

T = 2048
C = 1024
HD = 64
P = 128
TQ = 512
NTQ = T // TQ  # 4
NTK = T // P   # 16
QCOLS = 512

# wconst column offsets (bf16 elements per partition). Ordered so the
# first-needed bytes transfer first: ones/tri/wq/wk/wv in DMA part 1,
# wo (only needed ~40us in, at the first output projection) in part 2.
OFF_ONES = 0                    # 16
OFF_TRI = OFF_ONES + NTK        # 2 x 128 triangle mask
OFF_WQ = OFF_TRI + 2 * P        # 8 chunks x 512
OFF_WK = OFF_WQ + 8 * QCOLS     # 8 chunks x 128
OFF_WV = OFF_WK + 8 * 128       # 8 chunks x 128
OFF_WO = OFF_WV + 8 * 128       # 4 chunks x 1024
NCONST = OFF_WO + 4 * C         # 10512

_PROG = None


def _build_program():
    import concourse.mybir as mybir
    import concourse.tile as tile
    from concourse import bacc

    FP32 = mybir.dt.float32
    BF16 = mybir.dt.bfloat16
    AF = mybir.ActivationFunctionType
    ALU = mybir.AluOpType

    nc = bacc.Bacc("TRN2", target_bir_lowering=False, debug=False, num_devices=8)

    xT = nc.dram_tensor("xT", [C, T], BF16, kind="ExternalInput").ap()
    wconst = nc.dram_tensor("wconst", [P, NCONST], BF16, kind="ExternalInput").ap()
    y = nc.dram_tensor("y", [T, C], FP32, kind="ExternalOutput").ap()

    with tile.TileContext(nc) as tc:
        with tc.tile_pool(name="persist", bufs=1) as pp:
            # per-block K / Vext tiles (split so stage A(t+1) writes don't
            # WAR-serialize against stage B(t) reads)
            kTb = [pp.tile([P, TQ], BF16, tag=f"kT{t}", name=f"kT{t}")
                   for t in range(NTQ)]
            veA = [pp.tile([P, 4, HD + 1], BF16, tag=f"veA{t}", name=f"veA{t}")
                   for t in range(NTQ)]
            veB = [pp.tile([P, 4, HD + 1], BF16, tag=f"veB{t}", name=f"veB{t}")
                   for t in range(NTQ)]

            # weights + constants in two DMAs: part 1 (ones/tri/wq/wk/wv)
            # up front; part 2 (wo) issued after block 0's x tile so the
            # first projections aren't stuck behind wo's megabyte.
            wc = pp.tile([P, NCONST], BF16, tag="wc")
            nc.sync.dma_start(out=wc[:, :OFF_WO], in_=wconst[:, :OFF_WO])

            def wq_ap(k, f0=0, f1=QCOLS):
                return wc[:, OFF_WQ + QCOLS * k + f0:OFF_WQ + QCOLS * k + f1]

            def wk_ap(k):
                return wc[:, OFF_WK + 128 * k:OFF_WK + 128 * (k + 1)]

            def wv_ap(k):
                return wc[:, OFF_WV + 128 * k:OFF_WV + 128 * (k + 1)]

            def wo_ap(p, f0, f1):
                return wc[:, OFF_WO + C * p + f0:OFF_WO + C * p + f1]

            ones_ap = wc[:, OFF_ONES:OFF_ONES + NTK]
            tri_ap = wc[:, OFF_TRI:OFF_TRI + 2 * P].rearrange(
                "p (h q) -> p h q", h=2
            )

            # PSUM budget (8 banks): pv 3 x 1 bank (PV accumulators)
            # + pj 1 x 1 bank (projection accumulators) + s2 2 x 2 banks
            # (scores / output-projection tiles). Keeping the projection
            # ring separate from the PV ring is what lets stage A(t+1)
            # matmuls run while stage B(t) is still mid-flight.
            with tc.tile_pool(name="xw", bufs=2) as xw, \
                 tc.tile_pool(name="pvp", bufs=3, space="PSUM") as pvp, \
                 tc.tile_pool(name="pjp", bufs=1, space="PSUM") as pjp, \
                 tc.tile_pool(name="scps", bufs=2, space="PSUM") as scps, \
                 tc.tile_pool(name="ptsb", bufs=6) as ptsb, \
                 tc.tile_pool(name="blk", bufs=2) as blk, \
                 tc.tile_pool(name="nrm", bufs=2) as nrm:
                qTs_of = {}

                def stage_a(t):
                    """Projections for token block t -> qTs, kTb[t], ve*[t]."""
                    ts = slice(TQ * t, TQ * (t + 1))
                    xt = xw.tile([P, 8, TQ], BF16, tag="xt")
                    nc.sync.dma_start(
                        out=xt[:],
                        in_=xT[:, ts].rearrange("(k p) q -> p k q", p=P))
                    if t == 0:
                        nc.sync.dma_start(out=wc[:, OFF_WO:],
                                          in_=wconst[:, OFF_WO:])
                    qTs = [blk.tile([P, TQ], BF16, tag=f"q{f}", name=f"qt{f}")
                           for f in range(4)]
                    qTs_of[t] = qTs
                    for f in range(4):
                        ps = pjp.tile([P, TQ], FP32, tag="pj")
                        for k in range(8):
                            nc.tensor.matmul(
                                ps[:],
                                wq_ap(k, P * f, P * (f + 1)),
                                xt[:, k, :],
                                start=(k == 0), stop=(k == 7),
                            )
                        nc.vector.tensor_copy(qTs[f][:], ps[:])
                    ps = pjp.tile([P, TQ], FP32, tag="pj")
                    for k in range(8):
                        nc.tensor.matmul(ps[:], wk_ap(k), xt[:, k, :],
                                         start=(k == 0), stop=(k == 7))
                    nc.vector.tensor_copy(kTb[t][:], ps[:])

                    # V projection directly token-major: out[t, d2] with the
                    # x chunk as the stationary operand and wv as the moving
                    # one. No PE transpose / identity / vT staging needed.
                    vo = pjp.tile([P, TQ], FP32, tag="pj")
                    for sb in range(4):
                        for k in range(8):
                            nc.tensor.matmul(
                                vo[:, P * sb:P * (sb + 1)],
                                xt[:, k, P * sb:P * (sb + 1)],
                                wv_ap(k),
                                start=(k == 0), stop=(k == 7),
                            )
                    nc.vector.tensor_copy(veA[t][:, :, HD], ones_ap[:, 0:4])
                    nc.vector.tensor_copy(veB[t][:, :, HD], ones_ap[:, 0:4])
                    for sb in range(4):
                        nc.vector.tensor_copy(
                            veA[t][:, sb, 0:HD], vo[:, P * sb:P * sb + HD])
                        nc.vector.tensor_copy(
                            veB[t][:, sb, 0:HD],
                            vo[:, P * sb + HD:P * sb + 2 * HD])

                attnT_of = {}

                def stage_b(c):
                    """Causal attention for query block c."""
                    jmax = 4 * c + 3
                    qTs = qTs_of[c]
                    attnT = [blk.tile([P, TQ], BF16, tag=f"a{p}", name=f"at{p}")
                             for p in range(4)]
                    attnT_of[c] = attnT
                    for p in range(4):
                        # PV accumulators in their own 3-slot ring: oa(p+1)
                        # never waits; ob(p+1) waits only on normalize(p)'s
                        # first mul having drained oa(p)
                        oaT = pvp.tile([P, TQ], FP32, tag="pv", name="oaT")
                        obT = pvp.tile([P, TQ], FP32, tag="pv", name="obT")
                        outA = oaT[0:HD + 1, :]
                        outB = obT[0:HD + 1, :]
                        for j in range(jmax + 1):
                            r = j - 4 * c
                            # col restriction: cols < 128r are fully masked
                            moff = 128 * r if r >= 1 else 0
                            aoff = moff
                            tb, jj = j // 4, j % 4
                            s2 = scps.tile([P, 2, TQ], FP32, tag="s2")
                            nc.tensor.matmul(
                                s2[:, 0, moff:],
                                kTb[tb][0:64, P * jj:P * (jj + 1)],
                                qTs[p][0:64, moff:],
                                start=True, stop=True, tile_position=(0, 0),
                            )
                            nc.tensor.matmul(
                                s2[:, 1, moff:],
                                kTb[tb][64:128, P * jj:P * (jj + 1)],
                                qTs[p][64:128, moff:],
                                start=True, stop=True, tile_position=(64, 0),
                            )
                            pt = ptsb.tile([P, 2, TQ], BF16, tag="pt")
                            if r < 1:
                                # full tile: flattened AP -> one ACT inst
                                nc.scalar.activation(
                                    pt[:].rearrange("p h q -> p (h q)"),
                                    s2[:].rearrange("p h q -> p (h q)"),
                                    AF.Exp, scale=0.125)
                            else:
                                nc.scalar.activation(pt[:, :, aoff:],
                                                     s2[:, :, aoff:],
                                                     AF.Exp, scale=0.125)
                            if r >= 0:
                                # causal triangle of the diagonal 128x128
                                # sub-block: keep where tq_local - tk >= 0.
                                # DVE mask-mul (~0.3us) for r<2; gpsimd
                                # affine_select (~1.1us) for r>=2 to spread
                                # engine load.
                                if r < 2:
                                    nc.vector.tensor_mul(
                                        pt[:, :, aoff:aoff + 128],
                                        pt[:, :, aoff:aoff + 128],
                                        tri_ap,
                                    )
                                else:
                                    nc.gpsimd.affine_select(
                                        out=pt[:, :, aoff:aoff + 128],
                                        in_=pt[:, :, aoff:aoff + 128],
                                        compare_op=ALU.is_ge, fill=0.0,
                                        base=0,
                                        pattern=[[0, 2], [1, 128]],
                                        channel_multiplier=-1,
                                    )
                            # cols < aoff are fully masked: never computed,
                            # never read -- PV accumulates only [aoff:] (j=0
                            # is always a full block, so PSUM is fully
                            # initialized at start)
                            nc.tensor.matmul(outA[:, aoff:], veA[tb][:, jj, :],
                                             pt[:, 0, aoff:],
                                             start=(j == 0), stop=(j == jmax))
                            nc.tensor.matmul(outB[:, aoff:], veB[tb][:, jj, :],
                                             pt[:, 1, aoff:],
                                             start=(j == 0), stop=(j == jmax))
                        # normalize: recip the denominator row pair, broadcast
                        # over the pair's partitions, scale PV outs into attnT
                        da = nrm.tile([1, TQ], FP32, tag="da")
                        db = nrm.tile([1, TQ], FP32, tag="db")
                        nc.vector.tensor_copy(da[:], outA[64:65, :])
                        nc.vector.tensor_copy(db[:], outB[64:65, :])
                        dar = nrm.tile([1, TQ], FP32, tag="dar")
                        dbr = nrm.tile([1, TQ], FP32, tag="dbr")
                        nc.vector.reciprocal_approx_fast(dar[:], da[:])
                        nc.vector.reciprocal_approx_fast(dbr[:], db[:])
                        bcA = nrm.tile([64, TQ], FP32, tag="bcA")
                        bcB = nrm.tile([64, TQ], FP32, tag="bcB")
                        nc.gpsimd.partition_broadcast(bcA[:], dar[:])
                        nc.gpsimd.partition_broadcast(bcB[:], dbr[:])
                        nc.vector.tensor_mul(attnT[p][0:64, :], outA[0:64, :], bcA[:])
                        nc.vector.tensor_mul(attnT[p][64:128, :], outB[0:64, :], bcB[:])

                def stage_c(c):
                    """Output projection rows for query block c."""
                    attnT = attnT_of[c]
                    for tt in range(4 * c, 4 * c + 4):
                        jj = tt - 4 * c
                        yc = scps.tile([P, 2, TQ], FP32, tag="s2")
                        for co in range(2):
                            for p in range(4):
                                nc.tensor.matmul(
                                    yc[:, co, :],
                                    attnT[p][:, P * jj:P * (jj + 1)],
                                    wo_ap(p, TQ * co, TQ * (co + 1)),
                                    start=(p == 0), stop=(p == 3),
                                )
                        yt = ptsb.tile([P, 2, TQ], FP32, tag="yt", bufs=2)
                        nc.vector.tensor_copy(yt[:], yc[:])
                        nc.sync.dma_start(out=y[P * tt:P * (tt + 1), :], in_=yt[:])

                # software pipeline: A(t+1) emitted after B(t) so its
                # projection matmuls fill the PE while B(t) is exp-bound
                stage_a(0)
                for t in range(NTQ):
                    stage_b(t)
                    if t + 1 < NTQ:
                        stage_a(t + 1)
                    stage_c(t)

    nc.compile()
    return nc


def get_program():
    global _PROG
    if _PROG is None:
        _PROG = _build_program()
    return _PROG


def make_in_maps(x, Wq, Wk, Wv, Wo):
    """Build the per-core input dicts (host-side sharding + layout prep)."""
    import ml_dtypes
    bf16 = ml_dtypes.bfloat16
    x = np.asarray(x, np.float32)
    Wq = np.asarray(Wq, np.float32)
    Wk = np.asarray(Wk, np.float32)
    Wv = np.asarray(Wv, np.float32)
    Wo = np.asarray(Wo, np.float32)
    ones = np.ones((P, NTK), np.float32)
    tri_blk = np.greater_equal(
        np.arange(P)[None, :], np.arange(P)[:, None]
    ).astype(np.float32)  # [key p, query q]: 1 where q >= p
    tri = np.concatenate([tri_blk, tri_blk], axis=1)  # [128, 256]
    in_maps = []
    for core in range(8):
        b, half = core // 2, core % 2
        h0 = 8 * half
        kv0 = 2 * half
        # pair-permuted local head order: [h0, h0+4, h0+1, h0+5, ...]
        heads = []
        for p in range(4):
            heads += [h0 + p, h0 + p + 4]
        qrows = np.concatenate([Wq[h * HD:(h + 1) * HD] for h in heads], 0)  # [512, C]
        wocols = np.concatenate([Wo[:, h * HD:(h + 1) * HD] for h in heads], 1)  # [C, 512]
        qrowsT = np.ascontiguousarray(qrows.T)            # [1024, 512]
        wkT = np.ascontiguousarray(Wk[kv0 * HD:(kv0 + 2) * HD].T)  # [1024, 128]
        wvT = np.ascontiguousarray(Wv[kv0 * HD:(kv0 + 2) * HD].T)  # [1024, 128]
        woT = np.ascontiguousarray(wocols.T)              # [512, 1024]
        secs = (
            [ones, tri]
            + [qrowsT[P * k:P * (k + 1), :] for k in range(8)]
            + [wkT[P * k:P * (k + 1), :] for k in range(8)]
            + [wvT[P * k:P * (k + 1), :] for k in range(8)]
            + [woT[P * p:P * (p + 1), :] for p in range(4)]
        )
        wconst = np.concatenate(secs, axis=1)
        assert wconst.shape == (P, NCONST), wconst.shape
        in_maps.append({
            "xT": np.ascontiguousarray(x[b].T).astype(bf16),
            "wconst": wconst.astype(bf16),
        })
    return in_maps


def run_on_hw(in_maps, trace=False, **kw):
    from concourse.bass_utils import run_bass_kernel_spmd
    nc = get_program()
    return run_bass_kernel_spmd(nc, in_maps, list(range(8)), trace=trace, **kw)


def kernel(**inputs):
    in_maps = make_in_maps(
        inputs["x"], inputs["Wq"], inputs["Wk"], inputs["Wv"], inputs["Wo"]
    )
    res = run_on_hw(in_maps)
    out = np.empty((4, T, C), np.float32)
    for b in range(4):
        out[b] = res.results[2 * b]["y"] + res.results[2 * b + 1]["y"]
    return out


# revision 21
# speedup vs baseline: 1.0084x; 1.0084x over previous
"""GQA kernel for 8 TRN2 NeuronCores.

Model: B=4, T=2048, C=1024, 16 q heads / 4 kv heads / head_dim 64, causal.
Sharding: 16 (batch, kv-head-pair) units -> 2 per core. Core c handles batch
c//2 and kv-head pair (0,1) if c even else (2,3), i.e. q heads 0-7 or 8-15.
Each core computes its slice of the Q/K/V projections, local causal
attention, and a partial output projection (its 512 columns of the head
concat); the host sums the two partial y's per batch.

v6 (from ~305us v3 baseline; v4 = 283us):
- all weights + constants land in two DMAs (first-needed bytes first)
  instead of ~30 serial dma_starts (~0.6-1us fixed cost each).
- causal diagonal masking split DVE (r<2, tensor_mul with a 0/1 triangle
  tile, ~0.3us) / gpsimd affine_select (r>=2, ~1.1us) instead of all-gpsimd
  (134us busy in v3).
- r=3 score matmuls restricted to their live 128 query columns.
- V tiles in bf16 (PE transpose at 1 cycle/row).
- full-tile exps issued on flattened [128,1024] APs -> one ACT inst each.
- kT / vext split into PER-BLOCK tiles and stage emission reordered to
  B(t) -> A(t+1) -> C(t): A(t+1)'s writes no longer carry a whole-tile WAR
  hazard against B(t)'s reads, so the projection matmuls can fill the PE
  while B(t) is Scalar(exp)-bound.

Fused software pipeline over token blocks of 512: A(t) projections for
token block t -> B(c=t) attention for query block c (needs only k/v token
blocks <= c by causality) -> C(c) output projection rows for block c.

Attention per (pair p, query block c): head pair shares one qT tile
([128, 512] x 4: rows 0-63 head A, 64-127 head B). Scores for A and B are
two concurrent row-tiled K=64 matmuls into one 2-bank PSUM tile; one Exp
activation covers both. PV uses Vext=[V|1] (M=65) so the softmax
denominator falls out of row 64 of the PV accumulator; reciprocal via the
fast approx DVE op (~51 ULP) after a [1,512] copy to SBUF (custom DVE ops
can't read PSUM), then gpsimd partition_broadcast (dst must start at
partition 0).
"""

import numpy as np

T = 2048
C = 1024
HD = 64
P = 128
TQ = 512
NTQ = T // TQ  # 4
NTK = T // P   # 16
QCOLS = 512

# wconst column offsets (bf16 elements per partition). Ordered so the
# first-needed bytes transfer first: ones/tri/wq/wk/wv in DMA part 1,
# wo (only needed ~40us in, at the first output projection) in part 2.
OFF_ONES = 0                    # 16
OFF_TRI = OFF_ONES + NTK        # 2 x 128 triangle mask
OFF_WQ = OFF_TRI + 2 * P        # 8 chunks x 512
OFF_WK = OFF_WQ + 8 * QCOLS     # 8 chunks x 128
OFF_WV = OFF_WK + 8 * 128       # 8 chunks x 128
OFF_WO = OFF_WV + 8 * 128       # 4 chunks x 1024
NCONST = OFF_WO + 4 * C         # 10512

_PROG = None


def _build_program():
    import concourse.mybir as mybir
    import concourse.tile as tile
    from concourse import bacc

    FP32 = mybir.dt.float32
    BF16 = mybir.dt.bfloat16
    AF = mybir.ActivationFunctionType
    ALU = mybir.AluOpType

    nc = bacc.Bacc("TRN2", target_bir_lowering=False, debug=False, num_devices=8)

    xT = nc.dram_tensor("xT", [C, T], BF16, kind="ExternalInput").ap()
    wconst = nc.dram_tensor("wconst", [P, NCONST], BF16, kind="ExternalInput").ap()
    y = nc.dram_tensor("y", [T, C], FP32, kind="ExternalOutput").ap()

    with tile.TileContext(nc) as tc:
        with tc.tile_pool(name="persist", bufs=1) as pp:
            # per-block K / Vext tiles (split so stage A(t+1) writes don't
            # WAR-serialize against stage B(t) reads)
            kTb = [pp.tile([P, TQ], BF16, tag=f"kT{t}", name=f"kT{t}")
                   for t in range(NTQ)]
            veA = [pp.tile([P, 4, HD + 1], BF16, tag=f"veA{t}", name=f"veA{t}")
                   for t in range(NTQ)]
            veB = [pp.tile([P, 4, HD + 1], BF16, tag=f"veB{t}", name=f"veB{t}")
                   for t in range(NTQ)]

            # weights + constants in two DMAs: part 1 (ones/tri/wq/wk/wv)
            # up front; part 2 (wo) issued after block 0's x tile so the
            # first projections aren't stuck behind wo's megabyte.
            wc = pp.tile([P, NCONST], BF16, tag="wc")
            nc.sync.dma_start(out=wc[:, :OFF_WO], in_=wconst[:, :OFF_WO])

            def wq_ap(k, f0=0, f1=QCOLS):
                return wc[:, OFF_WQ + QCOLS * k + f0:OFF_WQ + QCOLS * k + f1]

            def wk_ap(k):
                return wc[:, OFF_WK + 128 * k:OFF_WK + 128 * (k + 1)]

            def wv_ap(k):
                return wc[:, OFF_WV + 128 * k:OFF_WV + 128 * (k + 1)]

            def wo_ap(p, f0, f1):
                return wc[:, OFF_WO + C * p + f0:OFF_WO + C * p + f1]

            ones_ap = wc[:, OFF_ONES:OFF_ONES + NTK]
            tri_ap = wc[:, OFF_TRI:OFF_TRI + 2 * P].rearrange(
                "p (h q) -> p h q", h=2
            )

            # PSUM budget (8 banks): pv 3 x 1 bank (PV accumulators)
            # + pj 1 x 1 bank (projection accumulators) + s2 2 x 2 banks
            # (scores / output-projection tiles). Keeping the projection
            # ring separate from the PV ring is what lets stage A(t+1)
            # matmuls run while stage B(t) is still mid-flight.
            with tc.tile_pool(name="xw", bufs=2) as xw, \
                 tc.tile_pool(name="pvp", bufs=3, space="PSUM") as pvp, \
                 tc.tile_pool(name="pjp", bufs=1, space="PSUM") as pjp, \
                 tc.tile_pool(name="scps", bufs=2, space="PSUM") as scps, \
                 tc.tile_pool(name="ptsb", bufs=6) as ptsb, \
                 tc.tile_pool(name="blk", bufs=2) as blk, \
                 tc.tile_pool(name="nrm", bufs=2) as nrm:
                qTs_of = {}

                def a_chunk_dma(t):
                    """Prefetch x for block t (and wo after block 0's x)."""
                    ts = slice(TQ * t, TQ * (t + 1))
                    xt = xw.tile([P, 8, TQ], BF16, tag="xt", name="xt")
                    nc.sync.dma_start(
                        out=xt[:],
                        in_=xT[:, ts].rearrange("(k p) q -> p k q", p=P))
                    if t == 0:
                        nc.sync.dma_start(out=wc[:, OFF_WO:],
                                          in_=wconst[:, OFF_WO:])
                    qTs_of[t] = [
                        blk.tile([P, TQ], BF16, tag=f"q{f}", name=f"qt{f}")
                        for f in range(4)]
                    return xt

                def a_chunk_q(t, xt, f):
                    ps = pjp.tile([P, TQ], FP32, tag="pj", name="ps")
                    for k in range(8):
                        nc.tensor.matmul(
                            ps[:],
                            wq_ap(k, P * f, P * (f + 1)),
                            xt[:, k, :],
                            start=(k == 0), stop=(k == 7),
                        )
                    nc.vector.tensor_copy(qTs_of[t][f][:], ps[:])

                def a_chunk_k(t, xt):
                    ps = pjp.tile([P, TQ], FP32, tag="pj", name="ps")
                    for k in range(8):
                        nc.tensor.matmul(ps[:], wk_ap(k), xt[:, k, :],
                                         start=(k == 0), stop=(k == 7))
                    nc.vector.tensor_copy(kTb[t][:], ps[:])

                def a_chunk_v(t, xt):
                    # V projection directly token-major: out[t, d2] with the
                    # x chunk as the stationary operand and wv as the moving
                    # one. No PE transpose / identity / vT staging needed.
                    vo = pjp.tile([P, TQ], FP32, tag="pj", name="vo")
                    for sb in range(4):
                        for k in range(8):
                            nc.tensor.matmul(
                                vo[:, P * sb:P * (sb + 1)],
                                xt[:, k, P * sb:P * (sb + 1)],
                                wv_ap(k),
                                start=(k == 0), stop=(k == 7),
                            )
                    nc.vector.tensor_copy(veA[t][:, :, HD], ones_ap[:, 0:4])
                    nc.vector.tensor_copy(veB[t][:, :, HD], ones_ap[:, 0:4])
                    for sb in range(4):
                        nc.vector.tensor_copy(
                            veA[t][:, sb, 0:HD], vo[:, P * sb:P * sb + HD])
                        nc.vector.tensor_copy(
                            veB[t][:, sb, 0:HD],
                            vo[:, P * sb + HD:P * sb + 2 * HD])

                def stage_a_chunks(t):
                    """Stage A as 4 filler chunks, to weave between B pairs."""
                    xt = a_chunk_dma(t)
                    return [
                        lambda: (a_chunk_q(t, xt, 0), a_chunk_q(t, xt, 1)),
                        lambda: (a_chunk_q(t, xt, 2), a_chunk_q(t, xt, 3)),
                        lambda: a_chunk_k(t, xt),
                        lambda: a_chunk_v(t, xt),
                    ]

                def stage_a(t):
                    for ch in stage_a_chunks(t):
                        ch()

                attnT_of = {}

                def stage_b(c, fillers=()):
                    """Causal attention for query block c. After each pair,
                    one filler chunk (next block's projections) is emitted so
                    its matmuls land in the PE stream where B is exp-bound."""
                    jmax = 4 * c + 3
                    qTs = qTs_of[c]
                    attnT = [blk.tile([P, TQ], BF16, tag=f"a{p}", name=f"at{p}")
                             for p in range(4)]
                    attnT_of[c] = attnT
                    for p in range(4):
                        # PV accumulators in their own 3-slot ring: oa(p+1)
                        # never waits; ob(p+1) waits only on normalize(p)'s
                        # first mul having drained oa(p)
                        oaT = pvp.tile([P, TQ], FP32, tag="pv", name="oaT")
                        obT = pvp.tile([P, TQ], FP32, tag="pv", name="obT")
                        outA = oaT[0:HD + 1, :]
                        outB = obT[0:HD + 1, :]
                        for j in range(jmax + 1):
                            r = j - 4 * c
                            # col restriction: cols < 128r are fully masked
                            moff = 128 * r if r >= 1 else 0
                            aoff = moff
                            tb, jj = j // 4, j % 4
                            s2 = scps.tile([P, 2, TQ], FP32, tag="s2")
                            nc.tensor.matmul(
                                s2[:, 0, moff:],
                                kTb[tb][0:64, P * jj:P * (jj + 1)],
                                qTs[p][0:64, moff:],
                                start=True, stop=True, tile_position=(0, 0),
                            )
                            nc.tensor.matmul(
                                s2[:, 1, moff:],
                                kTb[tb][64:128, P * jj:P * (jj + 1)],
                                qTs[p][64:128, moff:],
                                start=True, stop=True, tile_position=(64, 0),
                            )
                            pt = ptsb.tile([P, 2, TQ], BF16, tag="pt")
                            if r < 1:
                                # full tile: flattened AP -> one ACT inst
                                nc.scalar.activation(
                                    pt[:].rearrange("p h q -> p (h q)"),
                                    s2[:].rearrange("p h q -> p (h q)"),
                                    AF.Exp, scale=0.125)
                            else:
                                nc.scalar.activation(pt[:, :, aoff:],
                                                     s2[:, :, aoff:],
                                                     AF.Exp, scale=0.125)
                            if r >= 0:
                                # causal triangle of the diagonal 128x128
                                # sub-block: keep where tq_local - tk >= 0.
                                # DVE mask-mul (~0.3us) for r<2; gpsimd
                                # affine_select (~1.1us) for r>=2 to spread
                                # engine load.
                                if r < 2:
                                    nc.vector.tensor_mul(
                                        pt[:, :, aoff:aoff + 128],
                                        pt[:, :, aoff:aoff + 128],
                                        tri_ap,
                                    )
                                else:
                                    nc.gpsimd.affine_select(
                                        out=pt[:, :, aoff:aoff + 128],
                                        in_=pt[:, :, aoff:aoff + 128],
                                        compare_op=ALU.is_ge, fill=0.0,
                                        base=0,
                                        pattern=[[0, 2], [1, 128]],
                                        channel_multiplier=-1,
                                    )
                            # cols < aoff are fully masked: never computed,
                            # never read -- PV accumulates only [aoff:] (j=0
                            # is always a full block, so PSUM is fully
                            # initialized at start)
                            nc.tensor.matmul(outA[:, aoff:], veA[tb][:, jj, :],
                                             pt[:, 0, aoff:],
                                             start=(j == 0), stop=(j == jmax))
                            nc.tensor.matmul(outB[:, aoff:], veB[tb][:, jj, :],
                                             pt[:, 1, aoff:],
                                             start=(j == 0), stop=(j == jmax))
                        # normalize: recip the denominator row pair, broadcast
                        # over the pair's partitions, scale PV outs into attnT
                        da = nrm.tile([1, TQ], FP32, tag="da")
                        db = nrm.tile([1, TQ], FP32, tag="db")
                        nc.vector.tensor_copy(da[:], outA[64:65, :])
                        nc.vector.tensor_copy(db[:], outB[64:65, :])
                        dar = nrm.tile([1, TQ], FP32, tag="dar")
                        dbr = nrm.tile([1, TQ], FP32, tag="dbr")
                        nc.vector.reciprocal_approx_fast(dar[:], da[:])
                        nc.vector.reciprocal_approx_fast(dbr[:], db[:])
                        bcA = nrm.tile([64, TQ], FP32, tag="bcA")
                        bcB = nrm.tile([64, TQ], FP32, tag="bcB")
                        nc.gpsimd.partition_broadcast(bcA[:], dar[:])
                        nc.gpsimd.partition_broadcast(bcB[:], dbr[:])
                        nc.vector.tensor_mul(attnT[p][0:64, :], outA[0:64, :], bcA[:])
                        nc.vector.tensor_mul(attnT[p][64:128, :], outB[0:64, :], bcB[:])
                        if p < len(fillers):
                            fillers[p]()

                def stage_c(c):
                    """Output projection rows for query block c."""
                    attnT = attnT_of[c]
                    for tt in range(4 * c, 4 * c + 4):
                        jj = tt - 4 * c
                        yc = scps.tile([P, 2, TQ], FP32, tag="s2")
                        for co in range(2):
                            for p in range(4):
                                nc.tensor.matmul(
                                    yc[:, co, :],
                                    attnT[p][:, P * jj:P * (jj + 1)],
                                    wo_ap(p, TQ * co, TQ * (co + 1)),
                                    start=(p == 0), stop=(p == 3),
                                )
                        yt = ptsb.tile([P, 2, TQ], FP32, tag="yt", bufs=2)
                        nc.vector.tensor_copy(yt[:], yc[:])
                        nc.sync.dma_start(out=y[P * tt:P * (tt + 1), :], in_=yt[:])

                # software pipeline: A(t+1) woven between B(t)'s pairs so
                # its projection matmuls fill the PE while B(t) is exp-bound
                stage_a(0)
                for t in range(NTQ):
                    fillers = stage_a_chunks(t + 1) if t + 1 < NTQ else ()
                    stage_b(t, fillers)
                    stage_c(t)

    nc.compile()
    return nc


def get_program():
    global _PROG
    if _PROG is None:
        _PROG = _build_program()
    return _PROG


def make_in_maps(x, Wq, Wk, Wv, Wo):
    """Build the per-core input dicts (host-side sharding + layout prep)."""
    import ml_dtypes
    bf16 = ml_dtypes.bfloat16
    x = np.asarray(x, np.float32)
    Wq = np.asarray(Wq, np.float32)
    Wk = np.asarray(Wk, np.float32)
    Wv = np.asarray(Wv, np.float32)
    Wo = np.asarray(Wo, np.float32)
    ones = np.ones((P, NTK), np.float32)
    tri_blk = np.greater_equal(
        np.arange(P)[None, :], np.arange(P)[:, None]
    ).astype(np.float32)  # [key p, query q]: 1 where q >= p
    tri = np.concatenate([tri_blk, tri_blk], axis=1)  # [128, 256]
    in_maps = []
    for core in range(8):
        b, half = core // 2, core % 2
        h0 = 8 * half
        kv0 = 2 * half
        # pair-permuted local head order: [h0, h0+4, h0+1, h0+5, ...]
        heads = []
        for p in range(4):
            heads += [h0 + p, h0 + p + 4]
        qrows = np.concatenate([Wq[h * HD:(h + 1) * HD] for h in heads], 0)  # [512, C]
        wocols = np.concatenate([Wo[:, h * HD:(h + 1) * HD] for h in heads], 1)  # [C, 512]
        qrowsT = np.ascontiguousarray(qrows.T)            # [1024, 512]
        wkT = np.ascontiguousarray(Wk[kv0 * HD:(kv0 + 2) * HD].T)  # [1024, 128]
        wvT = np.ascontiguousarray(Wv[kv0 * HD:(kv0 + 2) * HD].T)  # [1024, 128]
        woT = np.ascontiguousarray(wocols.T)              # [512, 1024]
        secs = (
            [ones, tri]
            + [qrowsT[P * k:P * (k + 1), :] for k in range(8)]
            + [wkT[P * k:P * (k + 1), :] for k in range(8)]
            + [wvT[P * k:P * (k + 1), :] for k in range(8)]
            + [woT[P * p:P * (p + 1), :] for p in range(4)]
        )
        wconst = np.concatenate(secs, axis=1)
        assert wconst.shape == (P, NCONST), wconst.shape
        in_maps.append({
            "xT": np.ascontiguousarray(x[b].T).astype(bf16),
            "wconst": wconst.astype(bf16),
        })
    return in_maps


def run_on_hw(in_maps, trace=False, **kw):
    from concourse.bass_utils import run_bass_kernel_spmd
    nc = get_program()
    return run_bass_kernel_spmd(nc, in_maps, list(range(8)), trace=trace, **kw)


def kernel(**inputs):
    in_maps = make_in_maps(
        inputs["x"], inputs["Wq"], inputs["Wk"], inputs["Wv"], inputs["Wo"]
    )
    res = run_on_hw(in_maps)
    out = np.empty((4, T, C), np.float32)
    for b in range(4):
        out[b] = res.results[2 * b]["y"] + res.results[2 * b + 1]["y"]
    return out
